# revision 1
# baseline (speedup 1.0000x reference)
"""Trainium2 Bass kernel for a 2-layer GCN (Cora-style GNN message passing).

Computation (see reference):
    S1 = x @ W1                      # [N, 40]
    agg1[d] = sum_e w_e * S1[src_e]  (segment-sum over dst) + b1
    h = relu(agg1) * keep            # keep = (dropout_mask > 0.5) / 0.5
    S2 = h @ W2                      # [N, 7]
    agg2[d] = sum_e w_e * S2[src_e]  + b2
    out = log_softmax(agg2, axis=1)

Distribution (8 NeuronCores): nodes are sharded by dst range; each core owns
12,500 nodes (padded to 12,800) and all edges whose dst falls in its range.
Each core computes S1/S2 rows for its own nodes, the tables are all-gathered
(bf16), and the per-core segment-sum is an indirect-DMA gather of src rows
plus one-hot matmuls on the tensor engine:

  - edges are sorted by dst and packed into groups of 128 (partition dim),
    each group confined to a 16-dst window,
  - a host-precomputed "weighted one-hot" [128 edges, 16 slots] (bf16) holds
    w_e at the dst slot, so  onehot.T @ msg  scatter-adds 128 edges at once,
  - windows accumulate into a [16 slots, 8 windows, width] PSUM tile (PSUM
    partition offsets must be 32-aligned, so windows live in the free dim).

All group counts are unified across cores so the single SPMD program works
on every core; padding edges carry weight 0.
"""

import os
import numpy as np
import ml_dtypes
from dataclasses import dataclass

bf16 = ml_dtypes.bfloat16


@dataclass(frozen=True)
class Cfg:
    ncores: int = 8
    own: int = 12500          # real nodes per core
    nodes: int = 12800        # padded nodes per core (multiple of 128)
    feat: int = 1433
    fpad: int = 1536          # feat padded to multiple of 128
    hid: int = 40
    ncls: int = 7
    win: int = 32             # dst nodes per window (one-hot width)
    wpt: int = 4              # windows per 128-node tile (128/win)

    @property
    def tiles(self):
        return self.nodes // 128

    @property
    def windows(self):
        return self.nodes // self.win  # per core

    @property
    def kt(self):
        return self.fpad // 128

    @property
    def n(self):
        return self.ncores * self.own

    @property
    def table_rows(self):
        return self.ncores * self.nodes


CFG = Cfg()


# --------------------------------------------------------------------------
# Host-side preprocessing
# --------------------------------------------------------------------------

def host_prep(cfg, x, src, dst, edge_weight, W1, b1, W2, b2, dropout_mask_u):
    """Build per-core input arrays + the (core-invariant) group structure."""
    ncores, own, nodes, win = cfg.ncores, cfg.own, cfg.nodes, cfg.win

    src = src.astype(np.int64)
    dst = dst.astype(np.int64)
    # global table row of a src node (tables are concatenated per-core blocks
    # of `nodes` rows)
    src_row = (src // own) * nodes + (src % own)
    core = dst // own
    ldst = dst - core * own
    wloc = ldst // win                      # window within core [0, windows)
    slot = ldst - wloc * win                # [0, win)
    gwin = core * cfg.windows + wloc        # global window id

    nwin_total = ncores * cfg.windows
    cnt = np.bincount(gwin, minlength=nwin_total).reshape(ncores, cfg.windows)
    # unified groups-per-window across cores (>=1 so every psum slab is written)
    Gw = np.maximum(1, -(-cnt // 128)).max(axis=0)          # [windows]
    woff = np.concatenate([[0], np.cumsum(Gw)])             # group offsets
    G = int(woff[-1])

    # stable sort edges by global window; position within window
    order = np.argsort(gwin, kind="stable")
    gw_sorted = gwin[order]
    grp_start = np.concatenate(
        [[0], np.cumsum(np.bincount(gwin, minlength=nwin_total))]
    )
    pos_in_win = np.arange(len(src)) - grp_start[gw_sorted]
    # per-core padded edge position
    tgt = woff[gw_sorted % cfg.windows] * 128 + pos_in_win

    idx_cores = np.zeros((ncores, G * 128), np.int32)
    w_cores = np.zeros((ncores, G * 128), np.float32)
    slot_cores = np.zeros((ncores, G * 128), np.int64)
    c_sorted = gw_sorted // cfg.windows
    for k in range(ncores):
        m = c_sorted == k
        idx_cores[k, tgt[m]] = src_row[order[m]]
        w_cores[k, tgt[m]] = edge_weight[order[m]]
        slot_cores[k, tgt[m]] = slot[order[m]]

    # SBUF layouts: gidx [128, G] int32 ; onehot [128, G*win] bf16
    gidx = np.ascontiguousarray(
        idx_cores.reshape(ncores, G, 128).transpose(0, 2, 1)
    )
    oh = np.zeros((ncores, G * 128, win), np.float32)
    np.put_along_axis(oh, slot_cores[..., None], w_cores[..., None], axis=2)
    oh = np.ascontiguousarray(
        oh.reshape(ncores, G, 128, win).transpose(0, 2, 1, 3)
        .reshape(ncores, 128, G * win)
    ).astype(bf16)

    # per-core xT [fpad, nodes] bf16
    xT = np.zeros((ncores, cfg.fpad, nodes), bf16)
    for k in range(ncores):
        xT[k, : cfg.feat, :own] = x[k * own:(k + 1) * own].T.astype(bf16)

    # W1 packed [128, kt, hid] bf16
    w1p = np.zeros((cfg.fpad, cfg.hid), np.float32)
    w1p[: cfg.feat] = W1
    w1p = np.ascontiguousarray(
        w1p.reshape(cfg.kt, 128, cfg.hid).transpose(1, 0, 2)
    ).astype(bf16)

    # keep, transposed: [hid, nodes] f32
    keepT = np.zeros((ncores, cfg.hid, nodes), np.float32)
    keep_full = (dropout_mask_u > 0.5).astype(np.float32) * 2.0
    for k in range(ncores):
        keepT[k, :, :own] = keep_full[k * own:(k + 1) * own].T

    b1c = b1.astype(np.float32).reshape(cfg.hid, 1).copy()
    b2f = np.broadcast_to(
        b2.astype(np.float32), (cfg.win, cfg.wpt, cfg.ncls)
    ).copy()
    w2 = W2.astype(np.float32)

    in_maps = [
        {
            "xT": xT[k],
            "w1p": w1p,
            "w2": w2,
            "b1c": b1c,
            "b2f": b2f,
            "keepT": keepT[k],
            "gidx": gidx[k],
            "oh": oh[k],
        }
        for k in range(ncores)
    ]
    return in_maps, Gw


# --------------------------------------------------------------------------
# Numpy emulation of the device algorithm (for validation)
# --------------------------------------------------------------------------

def emulate(cfg, in_maps, Gw):
    f32 = np.float32
    ncores, nodes, win, hid, ncls = cfg.ncores, cfg.nodes, cfg.win, cfg.hid, cfg.ncls
    G = int(Gw.sum())
    woff = np.concatenate([[0], np.cumsum(Gw)])

    # phase A: S1 tables
    s1 = np.zeros((ncores, nodes, hid), bf16)
    for k in range(ncores):
        xT = in_maps[k]["xT"].astype(f32)           # [fpad, nodes]
        w1p = in_maps[k]["w1p"].astype(f32)         # [128, kt, hid]
        acc = np.zeros((nodes, hid), f32)
        for kk in range(cfg.kt):
            acc += xT[kk * 128:(kk + 1) * 128].T @ w1p[:, kk, :]
        s1[k] = acc.astype(bf16)
    s1_full = s1.reshape(ncores * nodes, hid)

    def spmm(table, width):
        aggs = np.zeros((ncores, nodes, width), f32)
        for k in range(ncores):
            gidx = in_maps[k]["gidx"]               # [128, G]
            oh = in_maps[k]["oh"].astype(f32)       # [128, G*win]
            msg = table[gidx.T.ravel()].astype(f32).reshape(G, 128, width)
            ohg = oh.reshape(128, G, win).transpose(1, 0, 2)  # [G,128,win]
            for w in range(cfg.windows):
                t, wl = divmod(w, cfg.wpt)
                base = t * 128 + wl * win
                for g in range(woff[w], woff[w + 1]):
                    aggs[k, base:base + win] += ohg[g].T @ msg[g]
        return aggs

    agg1 = spmm(s1_full, hid)
    s2 = np.zeros((ncores, nodes, ncls), bf16)
    for k in range(ncores):
        b1 = in_maps[k]["b1c"][:, 0]
        h = np.maximum(agg1[k] + b1, 0.0) * in_maps[k]["keepT"].T
        s2[k] = (h @ in_maps[k]["w2"]).astype(bf16)
    s2_full = s2.reshape(ncores * nodes, ncls)

    agg2 = spmm(s2_full, ncls)
    outs = []
    for k in range(ncores):
        z = agg2[k] + in_maps[k]["b2f"][0, 0]
        m = z.max(1, keepdims=True)
        out = (z - m) - np.log(np.exp(z - m).sum(1, keepdims=True))
        outs.append(out[: cfg.own])
    return np.concatenate(outs).astype(np.float32)


# --------------------------------------------------------------------------
# Bass/Tile program
# --------------------------------------------------------------------------

def build_program(cfg, Gw, num_devices):
    import concourse.bass as bass
    import concourse.bacc as bacc
    import concourse.mybir as mybir
    import concourse.tile as tile
    from concourse.masks import make_identity

    f32 = mybir.dt.float32
    bf = mybir.dt.bfloat16
    i32 = mybir.dt.int32
    AF = mybir.ActivationFunctionType
    OP = mybir.AluOpType
    X = mybir.AxisListType.X

    G = int(Gw.sum())
    woff = np.concatenate([[0], np.cumsum(Gw)])
    nodes, tiles, win, wpt = cfg.nodes, cfg.tiles, cfg.win, cfg.wpt
    hid, ncls, kt = cfg.hid, cfg.ncls, cfg.kt
    trows = num_devices * nodes

    nc = bacc.Bacc(
        "TRN2", target_bir_lowering=False, debug=False,
        num_devices=num_devices,
    )

    xT = nc.dram_tensor("xT", [cfg.fpad, nodes], bf, kind="ExternalInput")
    w1p = nc.dram_tensor("w1p", [128, kt, hid], bf, kind="ExternalInput")
    w2 = nc.dram_tensor("w2", [hid, ncls], f32, kind="ExternalInput")
    b1c = nc.dram_tensor("b1c", [hid, 1], f32, kind="ExternalInput")
    b2f = nc.dram_tensor("b2f", [win, wpt, ncls], f32, kind="ExternalInput")
    keepT = nc.dram_tensor("keepT", [hid, nodes], f32, kind="ExternalInput")
    gidx = nc.dram_tensor("gidx", [128, G], i32, kind="ExternalInput")
    oh = nc.dram_tensor("oh", [128, G * win], bf, kind="ExternalInput")
    out_d = nc.dram_tensor("out", [nodes, ncls], f32, kind="ExternalOutput")

    s1_own = nc.dram_tensor("s1_own", [nodes, hid], bf)
    s1_full = nc.dram_tensor("s1_full", [trows, hid], bf, addr_space="Shared")
    s2_own = nc.dram_tensor("s2_own", [nodes, ncls], bf)
    s2_full = nc.dram_tensor("s2_full", [trows, ncls], bf, addr_space="Shared")

    groups = list(range(num_devices))

    # per-tile group schedule: (g_global, window_in_tile, start, stop)
    sched = []
    for t in range(tiles):
        entries = []
        for wl in range(wpt):
            w = t * wpt + wl
            for j, g in enumerate(range(woff[w], woff[w + 1])):
                entries.append(
                    (int(g), wl, j == 0, g == woff[w + 1] - 1)
                )
        sched.append(entries)
    rmax = int(max(woff[(t + 1) * wpt] - woff[t * wpt] for t in range(tiles)))

    with tile.TileContext(nc) as tc:
        with (
            tc.tile_pool(name="const", bufs=1) as constp,
            tc.tile_pool(name="xbuf", bufs=3) as xpool,
            tc.tile_pool(name="psA", bufs=2, space="PSUM") as psA,
            tc.tile_pool(name="s1pc", bufs=3) as s1pool,
            tc.tile_pool(name="meta", bufs=3) as metap,
            tc.tile_pool(name="msg", bufs=3) as msgp,
            tc.tile_pool(name="psB", bufs=2, space="PSUM") as psB,
            tc.tile_pool(name="hb", bufs=3) as hpool,
            tc.tile_pool(name="psT", bufs=2, space="PSUM") as psT,
            tc.tile_pool(name="ps2", bufs=2, space="PSUM") as ps2,
            tc.tile_pool(name="ob", bufs=3) as opool,
        ):
            # ---- constants ----
            w1sb = constp.tile([128, kt, hid], bf)
            nc.sync.dma_start(out=w1sb[:], in_=w1p[:])
            w2sb = constp.tile([hid, ncls], f32)
            nc.sync.dma_start(out=w2sb[:], in_=w2[:])
            b1sb = constp.tile([hid, 1], f32)
            nc.sync.dma_start(out=b1sb[:], in_=b1c[:])
            b2sb = constp.tile([win, wpt, ncls], f32)
            nc.sync.dma_start(out=b2sb[:], in_=b2f[:])
            ident = constp.tile([128, 128], f32)
            make_identity(nc, ident[:])

            # ---- phase A: S1_own = (x @ W1) per 128-node chunk ----
            xT_r = xT[:].rearrange("(k p) n -> p k n", p=128)
            for c in range(tiles):
                xt = xpool.tile([128, kt, 128], bf)
                nc.sync.dma_start(
                    out=xt[:], in_=xT_r[:, :, c * 128:(c + 1) * 128]
                )
                ps = psA.tile([128, hid], f32)
                for k in range(kt):
                    nc.tensor.matmul(
                        ps[:], lhsT=xt[:, k, :], rhs=w1sb[:, k, :],
                        start=(k == 0), stop=(k == kt - 1),
                    )
                pc = s1pool.tile([128, hid], bf, tag="s1pc")
                nc.vector.tensor_copy(pc[:], ps[:])
                nc.sync.dma_start(
                    out=s1_own[c * 128:(c + 1) * 128, :], in_=pc[:]
                )

            # ---- all-gather S1 ----
            nc.gpsimd.collective_compute(
                "AllGather", OP.bypass, replica_groups=[groups],
                ins=[s1_own[:]], outs=[s1_full[:]],
            )

            # ---- layer 1 SpMM -> h^T -> S2_own ----
            for t in range(tiles):
                r0 = int(woff[t * wpt])
                rt = int(woff[(t + 1) * wpt]) - r0
                idxt = metap.tile([128, rmax], i32, tag="idx")
                nc.sync.dma_start(out=idxt[:, :rt], in_=gidx[:, r0:r0 + rt])
                oht = metap.tile([128, rmax, win], bf, tag="oh")
                nc.sync.dma_start(
                    out=oht[:, :rt, :],
                    in_=oh[:, r0 * win:(r0 + rt) * win]
                    .rearrange("p (r v) -> p r v", v=win),
                )
                msg = msgp.tile([128, rmax, hid], bf, tag="msg1")
                # funnel the gather's dependencies (idxt DMA, WAR on msg)
                # through cheap Pool-engine ops first
                scr = metap.tile([1, 1], i32, tag="scr")
                nc.gpsimd.tensor_copy(scr[:], idxt[:1, :1])
                nc.gpsimd.memset(msg[:1, :1, :1], 0.0)
                # HW only supports one offset per partition per indirect DMA
                for r in range(rt):
                    nc.gpsimd.indirect_dma_start(
                        out=msg[:, r, :], out_offset=None,
                        in_=s1_full[:],
                        in_offset=bass.IndirectOffsetOnAxis(
                            ap=idxt[:, r:r + 1], axis=0
                        ),
                    )
                ps = psB.tile([win, wpt, hid], f32, tag="agg")
                for (g, wl, st, sp) in sched[t]:
                    r = g - r0
                    nc.tensor.matmul(
                        ps[:, wl, :],
                        lhsT=oht[:, r, :], rhs=msg[:, r, :],
                        start=st, stop=sp,
                    )
                agg_sb = hpool.tile([win, wpt, hid], f32, tag="agg_sb")
                nc.vector.tensor_copy(agg_sb[:], ps[:])
                pst = psT.tile([hid, wpt, win], f32, tag="hT")
                for wl in range(wpt):
                    nc.tensor.transpose(
                        pst[:, wl, :], agg_sb[:, wl, :], ident[:win, :win]
                    )
                hT = hpool.tile([hid, 128], f32, tag="hT_sb")
                nc.scalar.activation(
                    out=hT[:],
                    in_=pst[:].rearrange("p w s -> p (w s)"),
                    func=AF.Relu, bias=b1sb[:], scale=1.0,
                )
                kpT = hpool.tile([hid, 128], f32, tag="keepT")
                nc.sync.dma_start(
                    out=kpT[:], in_=keepT[:, t * 128:(t + 1) * 128]
                )
                nc.vector.tensor_tensor(
                    out=hT[:], in0=hT[:], in1=kpT[:], op=OP.mult
                )
                p2 = ps2.tile([128, ncls], f32, tag="s2")
                nc.tensor.matmul(
                    p2[:], lhsT=hT[:], rhs=w2sb[:], start=True, stop=True
                )
                s2pc = s1pool.tile([128, ncls], bf, tag="s2pc")
                nc.vector.tensor_copy(s2pc[:], p2[:])
                nc.sync.dma_start(
                    out=s2_own[t * 128:(t + 1) * 128, :], in_=s2pc[:]
                )

            # ---- all-gather S2 ----
            nc.gpsimd.collective_compute(
                "AllGather", OP.bypass, replica_groups=[groups],
                ins=[s2_own[:]], outs=[s2_full[:]],
            )

            # ---- layer 2 SpMM + log_softmax ----
            out_r = out_d[:].rearrange("(t w s) c -> t s w c", s=win, w=wpt)
            for t in range(tiles):
                r0 = int(woff[t * wpt])
                rt = int(woff[(t + 1) * wpt]) - r0
                idxt = metap.tile([128, rmax], i32, tag="idx")
                nc.sync.dma_start(out=idxt[:, :rt], in_=gidx[:, r0:r0 + rt])
                oht = metap.tile([128, rmax, win], bf, tag="oh")
                nc.sync.dma_start(
                    out=oht[:, :rt, :],
                    in_=oh[:, r0 * win:(r0 + rt) * win]
                    .rearrange("p (r v) -> p r v", v=win),
                )
                msg = msgp.tile([128, rmax, ncls], bf, tag="msg2")
                scr = metap.tile([1, 1], i32, tag="scr")
                nc.gpsimd.tensor_copy(scr[:], idxt[:1, :1])
                nc.gpsimd.memset(msg[:1, :1, :1], 0.0)
                for r in range(rt):
                    nc.gpsimd.indirect_dma_start(
                        out=msg[:, r, :], out_offset=None,
                        in_=s2_full[:],
                        in_offset=bass.IndirectOffsetOnAxis(
                            ap=idxt[:, r:r + 1], axis=0
                        ),
                    )
                ps = psB.tile([win, wpt, ncls], f32, tag="agg")
                for (g, wl, st, sp) in sched[t]:
                    r = g - r0
                    nc.tensor.matmul(
                        ps[:, wl, :],
                        lhsT=oht[:, r, :], rhs=msg[:, r, :],
                        start=st, stop=sp,
                    )
                z = opool.tile([win, wpt, ncls], f32, tag="z")
                nc.vector.tensor_tensor(
                    out=z[:], in0=ps[:], in1=b2sb[:], op=OP.add
                )
                m = opool.tile([win, wpt], f32, tag="m")
                nc.vector.tensor_reduce(out=m[:], in_=z[:], axis=X, op=OP.max)
                zc = opool.tile([win, wpt, ncls], f32, tag="zc")
                nc.vector.tensor_tensor(
                    out=zc[:], in0=z[:],
                    in1=m[:].to_broadcast([win, wpt, ncls]), op=OP.subtract,
                )
                ez = opool.tile([win, wpt, ncls], f32, tag="ez")
                nc.scalar.activation(out=ez[:], in_=zc[:], func=AF.Exp)
                s = opool.tile([win, wpt], f32, tag="s")
                nc.vector.tensor_reduce(out=s[:], in_=ez[:], axis=X, op=OP.add)
                ls = opool.tile([win, wpt], f32, tag="ls")
                nc.scalar.activation(out=ls[:], in_=s[:], func=AF.Ln)
                res = opool.tile([win, wpt, ncls], f32, tag="res")
                nc.vector.tensor_tensor(
                    out=res[:], in0=zc[:],
                    in1=ls[:].to_broadcast([win, wpt, ncls]), op=OP.subtract,
                )
                nc.sync.dma_start(out=out_r[t], in_=res[:])

    nc.compile()
    return nc


# --------------------------------------------------------------------------
# Entry point
# --------------------------------------------------------------------------

def kernel(x, src, dst, edge_weight, W1, b1, W2, b2, dropout_mask_u):
    cfg = CFG
    in_maps, Gw = host_prep(
        cfg, x, src, dst, edge_weight, W1, b1, W2, b2, dropout_mask_u
    )
    nc = build_program(cfg, Gw, cfg.ncores)

    from concourse.bass_utils import run_bass_kernel_spmd

    trace = bool(int(os.environ.get("GNN_TRACE", "0")))
    try:
        res = run_bass_kernel_spmd(
            nc, in_maps, core_ids=list(range(cfg.ncores)), trace=trace
        )
    except ModuleNotFoundError:
        res = run_bass_kernel_spmd(
            nc, in_maps, core_ids=list(range(cfg.ncores)), trace=False
        )
    kernel.last_exec_time_ns = getattr(res, "exec_time_ns", None)
    kernel.last_profile = res
    out = np.concatenate(
        [res.results[k]["out"][: cfg.own] for k in range(cfg.ncores)]
    )
    return out.astype(np.float32)



# revision 16
# speedup vs baseline: 3.0382x; 3.0382x over previous
"""Trainium2 Bass kernel for a 2-layer GCN (Cora-style GNN message passing).

Computation (see reference):
    S1 = x @ W1                      # [N, 40]
    agg1[d] = sum_e w_e * S1[src_e]  (segment-sum over dst) + b1
    h = relu(agg1) * keep            # keep = (dropout_mask > 0.5) / 0.5
    out = log_softmax((A @ h) @ W2 + b2)   # reassociated: agg2 = A@h, then @W2

Distribution (8 NeuronCores): nodes are sharded by dst range; each core owns
12,500 nodes (padded to 12,800) and all edges whose dst falls in its range.
Each core computes S1 rows for its own nodes, the [102400, 40] bf16 tables
are all-gathered, and each per-core segment-sum is an indirect-DMA gather of
src rows plus one-hot matmuls on the tensor engine:

  - edges are sorted by dst and packed into groups of 128 (partition dim),
    each group confined to a 32-dst window,
  - the "weighted one-hot" [128 edges, 32 slots] bf16 is built ON DEVICE from
    per-edge (slot, weight) arrays via an is_equal + multiply on the DVE,
  - layer 1 accumulates node-major [32, 4, 40] PSUM tiles; layer 2 flips the
    matmul operands to produce hid-major [40, 4, 32] tiles that feed the
    final @W2 matmul directly (no tensor-engine transposes anywhere).

x is sent in natural [node, feat] layout as bf16 (cheap host bit-trick cast)
and transposed on device by the DMA xbar. All inputs are kept as small as
possible: the dominant cost in this environment is host->device transfer of
the inputs, not device execution. All group counts are unified across cores
so the single SPMD program works on every core; padding edges carry
weight 0.
"""

import os
import numpy as np
from dataclasses import dataclass


@dataclass(frozen=True)
class Cfg:
    ncores: int = 8
    own: int = 12500          # real nodes per core
    nodes: int = 12800        # padded nodes per core (multiple of 128)
    feat: int = 1433
    fpad: int = 1536          # feat padded to multiple of 128
    hid: int = 40
    ncls: int = 7
    win: int = 32             # dst nodes per window (one-hot width)
    wpt: int = 4              # windows per 128-node tile (128/win)
    grp: int = 512            # phase-A node group (per DMA-transpose batch)

    @property
    def tiles(self):
        return self.nodes // 128

    @property
    def windows(self):
        return self.nodes // self.win  # per core

    @property
    def kt(self):
        return self.fpad // 128

    @property
    def n(self):
        return self.ncores * self.own

    @property
    def table_rows(self):
        return self.ncores * self.nodes


CFG = Cfg()


# --------------------------------------------------------------------------
# Host-side preprocessing
# --------------------------------------------------------------------------

def _bf16_trunc_bits(a_f32):
    """bf16 bit pattern of a float32 array via truncation (no arithmetic).

    This numpy build has pathologically slow dtype-cast loops (~30 MB/s) but
    fast same-dtype strided copies, so all bf16 conversion is done with
    uint16 byte views. Truncation costs <=1 ulp (0.4% rel) vs round-to-
    nearest -- well within the error budget.
    """
    a = np.ascontiguousarray(a_f32, dtype=np.float32)
    return a.view(np.uint16).reshape(*a.shape[:-1], a.shape[-1] * 2)[
        ..., 1::2  # little-endian: high half-word of each f32
    ]


def host_prep(cfg, x, src, dst, edge_weight, W1, b1, W2, b2, dropout_mask_u):
    """Build per-core input arrays + the (core-invariant) group structure."""
    import ml_dtypes

    bf16 = ml_dtypes.bfloat16
    ncores, own, nodes, win, wpt = cfg.ncores, cfg.own, cfg.nodes, cfg.win, cfg.wpt
    windows, hid, tiles = cfg.windows, cfg.hid, cfg.tiles

    # ---- edge structure (sorted by dst window, packed into 128-edge groups)
    dst = np.ascontiguousarray(dst, dtype=np.int32)
    src = np.ascontiguousarray(src, dtype=np.int32)
    core = dst // own
    ldst = dst - core * own
    wloc = ldst // win
    slot = ldst - wloc * win                 # [0, win)
    gwin = core * windows + wloc             # global window id

    nwin_total = ncores * windows
    cnt_flat = np.bincount(gwin, minlength=nwin_total)
    cnt = cnt_flat.reshape(ncores, windows)
    Gw = np.maximum(1, -(-cnt // 128)).max(axis=0)          # [windows]
    woff = np.concatenate([[0], np.cumsum(Gw)]).astype(np.int64)
    G = int(woff[-1])

    order = np.argsort(gwin, kind="stable")
    gw_sorted = gwin[order]
    grp_start = np.concatenate([[0], np.cumsum(cnt_flat)])
    pos_in_win = np.arange(len(src), dtype=np.int64) - grp_start[gw_sorted]
    tgt = woff[gw_sorted % windows] * 128 + pos_in_win       # per-core slot
    c_sorted = gw_sorted // windows

    # table row of a src node (tables are per-core blocks of `nodes` rows)
    sc = src // own
    src_row = sc * nodes + (src - sc * own)

    flat = (c_sorted * (G * 128) + tgt).astype(np.int64)
    idx_all = np.zeros(ncores * G * 128, np.int32)
    idx_all[flat] = src_row[order]
    wgt_all = np.zeros(ncores * G * 128, np.uint16)
    wgt_all[flat] = _bf16_trunc_bits(edge_weight)[order]
    # slot values 0..31: bf16 bits = 0x4200 | ... ; build from a 32-entry LUT
    slot_lut = _bf16_trunc_bits(np.arange(win, dtype=np.float32)).copy()
    slot_all = np.zeros(ncores * G * 128, np.uint16)
    slot_all[flat] = slot_lut[slot[order]]

    gidx = np.ascontiguousarray(
        idx_all.reshape(ncores, G, 128).transpose(0, 2, 1))
    slotb = np.ascontiguousarray(
        slot_all.reshape(ncores, G, 128).transpose(0, 2, 1)).view(bf16)
    wgt = np.ascontiguousarray(
        wgt_all.reshape(ncores, G, 128).transpose(0, 2, 1)).view(bf16)

    # ---- x: natural layout, bf16 (truncation), padded
    xn = np.zeros((ncores, nodes, cfg.fpad), np.uint16)
    xn[:, :own, : cfg.feat] = _bf16_trunc_bits(x).reshape(ncores, own, cfg.feat)
    xn = xn.view(bf16)

    # ---- keep mask, reordered to [tiles, 32 slot, 4 win, 40] tile layout
    # keep value is 0.0 or 2.0; bf16 bits of 2.0 are 0x4000
    kp = np.zeros((ncores, nodes, hid), np.uint16)
    kbits = (dropout_mask_u > 0.5).astype(np.uint16)
    kbits <<= 14
    kp[:, :own] = kbits.reshape(ncores, own, hid)
    keep4 = np.ascontiguousarray(
        kp.reshape(ncores, tiles, wpt, win, hid).transpose(0, 1, 3, 2, 4)
    ).reshape(ncores, tiles, win, wpt * hid).view(bf16)

    # ---- weights / consts (small; any cast path is fine)
    w1pad = np.zeros((cfg.fpad, hid), np.float32)
    w1pad[: cfg.feat] = W1
    w1p = np.ascontiguousarray(
        w1pad.reshape(cfg.kt, 128, hid).transpose(1, 0, 2)
    ).astype(bf16)
    w2 = W2.astype(np.float32)
    b1r = np.broadcast_to(
        b1.astype(np.float32), (win, 1, hid)).copy()
    b2r = np.broadcast_to(
        b2.astype(np.float32), (128, 1, cfg.ncls)).copy()
    vslot = np.broadcast_to(
        np.arange(win, dtype=np.float32).astype(bf16), (128, 1, win)).copy()

    in_maps = [
        {
            "xn": xn[k],
            "w1p": w1p,
            "w2": w2,
            "b1r": b1r,
            "b2r": b2r,
            "vslot": vslot,
            "keep4": keep4[k],
            "gidx": gidx[k],
            "slotb": slotb[k],
            "wgt": wgt[k],
        }
        for k in range(ncores)
    ]
    return in_maps, Gw


# --------------------------------------------------------------------------
# Numpy emulation of the device algorithm (for validation)
# --------------------------------------------------------------------------

def emulate(cfg, in_maps, Gw):
    import ml_dtypes
    f32, f16 = np.float32, ml_dtypes.bfloat16
    ncores, nodes, win, wpt = cfg.ncores, cfg.nodes, cfg.win, cfg.wpt
    hid, ncls, tiles = cfg.hid, cfg.ncls, cfg.tiles
    G = int(Gw.sum())
    woff = np.concatenate([[0], np.cumsum(Gw)])

    # phase A: S1 tables (natural row order)
    s1 = np.zeros((ncores, nodes, hid), f16)
    for k in range(ncores):
        xk = in_maps[k]["xn"].astype(f32)     # [nodes, fpad]
        w1p = in_maps[k]["w1p"].astype(f32)   # [128, kt, hid]
        w1 = w1p.transpose(1, 0, 2).reshape(cfg.fpad, hid)
        s1[k] = (xk @ w1).astype(f16)
    s1_full = s1.reshape(ncores * nodes, hid)

    def build_onehot(k):
        slotb = in_maps[k]["slotb"].astype(f32)   # [128, G]
        wgt = in_maps[k]["wgt"].astype(f32)       # [128, G]
        oh = (slotb[:, :, None] == np.arange(win)[None, None, :])
        return (oh * wgt[:, :, None]).astype(f16).astype(f32)  # [128, G, win]

    def spmm(table, k, oh):
        gidx = in_maps[k]["gidx"]                 # [128, G]
        msg = table[gidx.T].astype(f32)           # [G, 128, hid]
        ohg = oh.transpose(1, 0, 2)               # [G, 128, win]
        agg = np.zeros((tiles, win, wpt, hid), f32)
        for w in range(cfg.windows):
            t, wl = divmod(w, wpt)
            for g in range(woff[w], woff[w + 1]):
                agg[t, :, wl, :] += ohg[g].T @ msg[g]
        return agg                                # [tiles, 32s, 4w, hid]

    h = np.zeros((ncores, nodes, hid), f16)
    for k in range(ncores):
        oh = build_onehot(k)
        agg1 = spmm(s1_full, k, oh)
        b1 = in_maps[k]["b1r"][0, 0]
        keep = in_maps[k]["keep4"].reshape(tiles, win, wpt, hid)
        hb = np.maximum(agg1 + b1, 0.0).astype(f16).astype(f32) * keep
        # natural row order: node (t, w, s) lives at hb[t, s, w]
        h[k] = hb.transpose(0, 2, 1, 3).reshape(nodes, hid).astype(f16)
        in_maps[k]["_oh"] = oh
    h_full = h.reshape(ncores * nodes, hid)

    outs = []
    for k in range(ncores):
        agg2 = spmm(h_full, k, in_maps[k]["_oh"])   # [tiles, 32s, 4w, hid]
        # natural node order: node (t, w, s) -> agg2[t, s, w]
        aggn = agg2.transpose(0, 2, 1, 3).reshape(nodes, hid)
        z = aggn @ in_maps[k]["w2"] + in_maps[k]["b2r"][0, 0]
        m = z.max(1, keepdims=True)
        out = (z - m) - np.log(np.exp(z - m).sum(1, keepdims=True))
        outs.append(out[: cfg.own])
        del in_maps[k]["_oh"]
    return np.concatenate(outs).astype(np.float32)


# --------------------------------------------------------------------------
# Bass/Tile program
# --------------------------------------------------------------------------

def build_program(cfg, Gw, num_devices):
    import concourse.bass as bass
    import concourse.bacc as bacc
    import concourse.mybir as mybir
    import concourse.tile as tile

    f32 = mybir.dt.float32
    bf = mybir.dt.bfloat16
    i32 = mybir.dt.int32
    AF = mybir.ActivationFunctionType
    OP = mybir.AluOpType
    X = mybir.AxisListType.X

    G = int(Gw.sum())
    woff = np.concatenate([[0], np.cumsum(Gw)])
    nodes, tiles, win, wpt = cfg.nodes, cfg.tiles, cfg.win, cfg.wpt
    hid, ncls, kt, grp = cfg.hid, cfg.ncls, cfg.kt, cfg.grp
    trows = num_devices * nodes

    nc = bacc.Bacc(
        "TRN2", target_bir_lowering=False, debug=False,
        num_devices=num_devices,
    )

    xn = nc.dram_tensor("xn", [nodes, cfg.fpad], bf, kind="ExternalInput")
    w1p = nc.dram_tensor("w1p", [128, kt, hid], bf, kind="ExternalInput")
    w2 = nc.dram_tensor("w2", [hid, ncls], f32, kind="ExternalInput")
    b1r = nc.dram_tensor("b1r", [win, 1, hid], f32, kind="ExternalInput")
    b2r = nc.dram_tensor("b2r", [128, 1, ncls], f32, kind="ExternalInput")
    vslot = nc.dram_tensor("vslot", [128, 1, win], bf, kind="ExternalInput")
    keep4 = nc.dram_tensor(
        "keep4", [tiles, win, wpt * hid], bf, kind="ExternalInput")
    gidx = nc.dram_tensor("gidx", [128, G], i32, kind="ExternalInput")
    slotb = nc.dram_tensor("slotb", [128, G], bf, kind="ExternalInput")
    wgt = nc.dram_tensor("wgt", [128, G], bf, kind="ExternalInput")
    out_d = nc.dram_tensor("out", [nodes, ncls], f32, kind="ExternalOutput")

    s1_own = nc.dram_tensor("s1_own", [nodes, hid], bf)
    s1_full = nc.dram_tensor("s1_full", [trows, hid], bf, addr_space="Shared")
    h_own = nc.dram_tensor("h_own", [nodes, hid], bf)
    h_full = nc.dram_tensor("h_full", [trows, hid], bf, addr_space="Shared")

    groups = list(range(num_devices))

    # per-tile group schedule: (g_global, window_in_tile, start, stop)
    sched = []
    for t in range(tiles):
        entries = []
        for wl in range(wpt):
            w = t * wpt + wl
            for g in range(woff[w], woff[w + 1]):
                entries.append(
                    (int(g), wl, g == woff[w], g == woff[w + 1] - 1)
                )
        sched.append(entries)
    rmax = int(max(woff[(t + 1) * wpt] - woff[t * wpt] for t in range(tiles)))

    ngrp = nodes // grp
    spg = grp // 128  # 128-node subtiles per phase-A group

    with tile.TileContext(nc) as tc:
        with (
            tc.tile_pool(name="const", bufs=1) as constp,
            tc.tile_pool(name="xbuf", bufs=3) as xpool,
            tc.tile_pool(name="psA", bufs=2, space="PSUM") as psA,
            tc.tile_pool(name="s1pc", bufs=3) as spool,
            tc.tile_pool(name="msg", bufs=3) as msgp,
            tc.tile_pool(name="oh", bufs=3) as ohp,
            tc.tile_pool(name="psB", bufs=2, space="PSUM") as psB,
            tc.tile_pool(name="hb", bufs=3) as hpool,
            tc.tile_pool(name="psC", bufs=2, space="PSUM") as psC,
            tc.tile_pool(name="ps2", bufs=2, space="PSUM") as ps2,
            tc.tile_pool(name="ob", bufs=3) as opool,
        ):
            # ---- constants + resident metadata ----
            w1sb = constp.tile([128, kt, hid], bf)
            nc.sync.dma_start(out=w1sb[:], in_=w1p[:])
            w2sb = constp.tile([hid, ncls], f32)
            nc.sync.dma_start(out=w2sb[:], in_=w2[:])
            b1sb = constp.tile([win, 1, hid], f32)
            nc.sync.dma_start(out=b1sb[:], in_=b1r[:])
            b2sb = constp.tile([128, 1, ncls], f32)
            nc.sync.dma_start(out=b2sb[:], in_=b2r[:])
            vs = constp.tile([128, 1, win], bf)
            nc.sync.dma_start(out=vs[:], in_=vslot[:])
            gix = constp.tile([128, G], i32)
            nc.sync.dma_start(out=gix[:], in_=gidx[:])
            slb = constp.tile([128, G], bf)
            nc.sync.dma_start(out=slb[:], in_=slotb[:])
            wgb = constp.tile([128, G], bf)
            nc.sync.dma_start(out=wgb[:], in_=wgt[:])

            # ---- phase A: S1_own = (x @ W1), row-permuted store ----
            for c in range(ngrp):
                xt = xpool.tile([128, kt, grp], bf)
                for k in range(kt):
                    nc.sync.dma_start(
                        out=xt[:, k, :],
                        in_=xn[c * grp:(c + 1) * grp, k * 128:(k + 1) * 128],
                        transpose=True,
                    )
                for sub in range(spg):
                    ps = psA.tile([128, hid], f32)
                    for k in range(kt):
                        nc.tensor.matmul(
                            ps[:],
                            lhsT=xt[:, k, sub * 128:(sub + 1) * 128],
                            rhs=w1sb[:, k, :],
                            start=(k == 0), stop=(k == kt - 1),
                        )
                    pc = spool.tile([128, hid], bf, tag="s1pc")
                    nc.vector.tensor_copy(pc[:], ps[:])
                    t_ = c * spg + sub
                    nc.sync.dma_start(
                        out=s1_own[t_ * 128:(t_ + 1) * 128, :], in_=pc[:]
                    )

            # ---- all-gather S1 ----
            nc.gpsimd.collective_compute(
                "AllGather", OP.bypass, replica_groups=[groups],
                ins=[s1_own[:]], outs=[s1_full[:]],
            )

            def gather_and_onehot(t, table, mtag, otag):
                r0 = int(woff[t * wpt])
                rt = int(woff[(t + 1) * wpt]) - r0
                msg = msgp.tile([128, rmax, hid], bf, tag=mtag)
                # funnel the gather's dependencies (WAR on msg) through
                # cheap Pool-engine ops first
                scr = spool.tile([1, 1], i32, tag="scr")
                nc.gpsimd.tensor_copy(scr[:], gix[:1, :1])
                nc.gpsimd.memset(msg[:1, :1, :1], 0.0)
                # HW only supports one offset per partition per indirect DMA
                for r in range(rt):
                    nc.gpsimd.indirect_dma_start(
                        out=msg[:, r, :], out_offset=None,
                        in_=table[:],
                        in_offset=bass.IndirectOffsetOnAxis(
                            ap=gix[:, r0 + r:r0 + r + 1], axis=0
                        ),
                    )
                oht = ohp.tile([128, rmax, win], bf, tag=otag)
                nc.vector.tensor_tensor(
                    out=oht[:, :rt, :],
                    in0=slb[:, r0:r0 + rt].to_broadcast([128, rt, win]),
                    in1=vs[:].to_broadcast([128, rt, win]),
                    op=OP.is_equal,
                )
                nc.vector.tensor_tensor(
                    out=oht[:, :rt, :],
                    in0=oht[:, :rt, :],
                    in1=wgb[:, r0:r0 + rt].to_broadcast([128, rt, win]),
                    op=OP.mult,
                )
                return r0, msg, oht

            # ---- layer 1 SpMM -> h (node-major psum) ----
            for t in range(tiles):
                r0, msg, oht = gather_and_onehot(t, s1_full, "msg1", "oh1")
                ps = psB.tile([win, wpt, hid], f32, tag="agg")
                for (g, wl, st, sp) in sched[t]:
                    r = g - r0
                    nc.tensor.matmul(
                        ps[:, wl, :],
                        lhsT=oht[:, r, :], rhs=msg[:, r, :],
                        start=st, stop=sp,
                    )
                hb = hpool.tile([win, wpt, hid], f32, tag="hb")
                nc.vector.tensor_tensor(
                    out=hb[:], in0=ps[:],
                    in1=b1sb[:].to_broadcast([win, wpt, hid]), op=OP.add,
                )
                nc.scalar.activation(out=hb[:], in_=hb[:], func=AF.Relu)
                kp = hpool.tile([win, wpt * hid], bf, tag="kp")
                nc.sync.dma_start(out=kp[:], in_=keep4[t])
                hf = hpool.tile([win, wpt, hid], bf, tag="hf")
                nc.vector.tensor_tensor(
                    out=hf[:], in0=hb[:],
                    in1=kp[:].rearrange("p (w c) -> p w c", w=wpt),
                    op=OP.mult,
                )
                nc.sync.dma_start(
                    out=h_own[t * 128:(t + 1) * 128, :]
                    .rearrange("(w s) c -> s w c", w=wpt, s=win),
                    in_=hf[:],
                )

            # ---- all-gather h ----
            nc.gpsimd.collective_compute(
                "AllGather", OP.bypass, replica_groups=[groups],
                ins=[h_own[:]], outs=[h_full[:]],
            )

            # ---- layer 2 SpMM (hid-major psum) + @W2 + log_softmax ----
            for t in range(tiles):
                r0, msg, oht = gather_and_onehot(t, h_full, "msg2", "oh2")
                pst = psC.tile([hid, wpt, win], f32, tag="aggT")
                for (g, wl, st, sp) in sched[t]:
                    r = g - r0
                    nc.tensor.matmul(
                        pst[:, wl, :],
                        lhsT=msg[:, r, :], rhs=oht[:, r, :],
                        start=st, stop=sp,
                    )
                at = hpool.tile([hid, wpt, win], f32, tag="at")
                nc.vector.tensor_copy(at[:], pst[:])
                p2 = ps2.tile([128, 1, ncls], f32, tag="s2")
                nc.tensor.matmul(
                    p2[:, 0, :], lhsT=at[:].rearrange("p w s -> p (w s)"),
                    rhs=w2sb[:], start=True, stop=True,
                )
                z = opool.tile([128, 1, ncls], f32, tag="z")
                nc.vector.tensor_tensor(
                    out=z[:], in0=p2[:], in1=b2sb[:], op=OP.add,
                )
                m = opool.tile([128, 1], f32, tag="m")
                nc.vector.tensor_reduce(out=m[:], in_=z[:], axis=X, op=OP.max)
                zc = opool.tile([128, 1, ncls], f32, tag="zc")
                nc.vector.tensor_tensor(
                    out=zc[:], in0=z[:],
                    in1=m[:].to_broadcast([128, 1, ncls]), op=OP.subtract,
                )
                ez = opool.tile([128, 1, ncls], f32, tag="ez")
                nc.scalar.activation(out=ez[:], in_=zc[:], func=AF.Exp)
                s = opool.tile([128, 1], f32, tag="s")
                nc.vector.tensor_reduce(out=s[:], in_=ez[:], axis=X, op=OP.add)
                ls = opool.tile([128, 1], f32, tag="ls")
                nc.scalar.activation(out=ls[:], in_=s[:], func=AF.Ln)
                res = opool.tile([128, 1, ncls], f32, tag="res")
                nc.vector.tensor_tensor(
                    out=res[:], in0=zc[:],
                    in1=ls[:].to_broadcast([128, 1, ncls]), op=OP.subtract,
                )
                nc.sync.dma_start(
                    out=out_d[t * 128:(t + 1) * 128, :], in_=res[:, 0, :]
                )

    nc.compile()
    return nc


# --------------------------------------------------------------------------
# Entry point
# --------------------------------------------------------------------------

def kernel(x, src, dst, edge_weight, W1, b1, W2, b2, dropout_mask_u):
    cfg = CFG
    in_maps, Gw = host_prep(
        cfg, x, src, dst, edge_weight, W1, b1, W2, b2, dropout_mask_u
    )
    nc = build_program(cfg, Gw, cfg.ncores)

    from concourse.bass_utils import run_bass_kernel_spmd

    trace = bool(int(os.environ.get("GNN_TRACE", "0")))
    try:
        res = run_bass_kernel_spmd(
            nc, in_maps, core_ids=list(range(cfg.ncores)), trace=trace
        )
    except ModuleNotFoundError:
        res = run_bass_kernel_spmd(
            nc, in_maps, core_ids=list(range(cfg.ncores)), trace=False
        )
    kernel.last_exec_time_ns = getattr(res, "exec_time_ns", None)
    kernel.last_profile = res
    kernel.last_nc = nc
    kernel.last_in_maps = in_maps
    out = np.concatenate(
        [res.results[k]["out"][: cfg.own] for k in range(cfg.ncores)]
    )
    return out.astype(np.float32)
